# revision 25
# baseline (speedup 1.0000x reference)
"""Trainium2 Bass kernel for nn_AutoEncoder_17695265259991.

Set-transformer autoencoder over 4096 ragged sets (sizes alternating 32/64,
196608 flat rows, DIM=256, W=32), data-parallel over 8 NeuronCores.

Per core: 24576 rows = 256 periods of 96 rows (one 32-set + one 64-set).
Rows are regrouped into 128-row windows holding whole sets only:
  A-windows: two 64-sets   (128 of them per core)
  B-windows: four 32-sets  (64 of them per core)
Windows are processed in pairs (shared PSUM banks, paired evacuations) and
DMAed in batches of 16 windows (2 MiB per transfer).

Per-window pipeline (all f32, PSUM accumulation):
  x [128,256] --PE transpose--> xT [256p,128] --mm Wkv--> [Yk^T;Yv^T] [64,128]
  (+ sinusoid-PE pattern folded in as a constant added during PSUM evacuation)
  C^T = Yv^T.T @ Q^T(pos-pattern)  -> masked block-diagonal (constant mask)
  D^T = Yk_nat.T @ C^T ; out = [D^T;1].T @ [Wmap^T;bmap]  -> [128,256] rows
"""

import numpy as np

DIM = 256
W = 32
MAXN = 64
N_CORES = 8
N_ROWS = 196608
PERIOD = 96                      # 32-set rows + 64-set rows
ROWS_PER_CORE = N_ROWS // N_CORES          # 24576
PERIODS_PER_CORE = ROWS_PER_CORE // PERIOD  # 256
WB = 16                          # windows per DMA batch


def sinusoid_pe(length, dim):
    pos = np.arange(length, dtype=np.float32)[:, None]
    i = np.arange(dim)[None, :]
    angle = pos / np.power(10000.0, (2.0 * (i // 2)).astype(np.float32) / dim)
    return np.where(i % 2 == 0, np.sin(angle), np.cos(angle)).astype(np.float32)


def _block_diag_mask(sizes):
    R = sum(sizes)
    m = np.zeros((R, R), np.float32)
    off = 0
    for n in sizes:
        m[off:off + n, off:off + n] = 1.0
        off += n
    return m


def make_consts(Wk, Wv, Wq, Wmap, bmap):
    """Host-side packing of the tiny weight-derived constants."""
    pe = sinusoid_pe(MAXN, DIM)
    wkv = np.concatenate([Wk.T, Wv.T], axis=1).astype(np.float32)     # [256, 64]
    # SBUF layout [128, 128]: cols 0:64 = dims 0:128, cols 64:128 = dims 128:256
    wkv_s = np.concatenate([wkv[0:128], wkv[128:256]], axis=1)        # [128, 128]
    Q = (pe @ Wq.T).astype(np.float32)                                # [64, 32]
    pos_a = np.concatenate([np.arange(64), np.arange(64)])
    pos_b = np.concatenate([np.arange(32)] * 4)
    pkv_a = (pe[pos_a] @ wkv).T.astype(np.float32)                    # [64, 128]
    pkv_b = (pe[pos_b] @ wkv).T.astype(np.float32)
    # Q^T patterns live at partitions 32:64 so their base partition matches
    # the Yv^T slice of the stacked [Yk^T;Yv^T] tile used as matmul lhsT.
    qr_a = np.zeros((64, 128), np.float32)
    qr_a[32:64] = Q[pos_a].T
    qr_b = np.zeros((64, 128), np.float32)
    qr_b[32:64] = Q[pos_b].T
    mask_a = _block_diag_mask([64, 64])
    mask_b = _block_diag_mask([32, 32, 32, 32])
    wmapb = np.concatenate([Wmap.T, bmap[None]], axis=0).astype(np.float32)  # [33, 256]
    ident = np.eye(128, dtype=np.float32)
    ones = np.ones((1, 256), dtype=np.float32)
    # pkv/mask are stored pair-wide (2 windows side by side)
    return dict(wkv=wkv_s,
                pkv_a=np.tile(pkv_a, (1, 2)), pkv_b=np.tile(pkv_b, (1, 2)),
                qr_a=qr_a, qr_b=qr_b,
                mask_a=np.tile(mask_a, (1, 2)), mask_b=np.tile(mask_b, (1, 2)),
                wmapb=wmapb, ident=ident, ones=ones)


def build_nc(periods=PERIODS_PER_CORE):
    """Build the per-core Bass program. `periods` must be divisible by 4*WB."""
    import concourse.bass as bass
    import concourse.mybir as mybir
    from concourse import bacc, tile

    DT = mybir.dt.float32
    DTR = mybir.dt.float32r
    assert periods % (4 * WB) == 0, periods
    rows = periods * PERIOD
    n_ab = periods // 2 // WB      # A batches (WB windows each)
    n_bb = periods // 4 // WB      # B batches

    nc = bacc.Bacc()
    x_d = nc.dram_tensor("x", (rows, DIM), DTR, kind="ExternalInput")
    out_d = nc.dram_tensor("out", (rows, DIM), DT, kind="ExternalOutput")
    wkv_d = nc.dram_tensor("wkv", (128, 128), DTR, kind="ExternalInput")
    pkv_a_d = nc.dram_tensor("pkv_a", (64, 256), DT, kind="ExternalInput")
    pkv_b_d = nc.dram_tensor("pkv_b", (64, 256), DT, kind="ExternalInput")
    qr_a_d = nc.dram_tensor("qr_a", (64, 128), DTR, kind="ExternalInput")
    qr_b_d = nc.dram_tensor("qr_b", (64, 128), DTR, kind="ExternalInput")
    mask_a_d = nc.dram_tensor("mask_a", (128, 256), DT, kind="ExternalInput")
    mask_b_d = nc.dram_tensor("mask_b", (128, 256), DT, kind="ExternalInput")
    wmapb_d = nc.dram_tensor("wmapb", (33, 256), DTR, kind="ExternalInput")
    ident_d = nc.dram_tensor("ident", (128, 128), DTR, kind="ExternalInput")
    ones_d = nc.dram_tensor("ones", (1, 256), DTR, kind="ExternalInput")

    x3 = x_d.rearrange("(n r) c -> n r c", r=PERIOD)
    o3 = out_d.rearrange("(n r) c -> n r c", r=PERIOD)

    with tile.TileContext(nc) as tc:
        with (
            tc.tile_pool(name="const", bufs=1) as cpool,
            tc.tile_pool(name="xn", bufs=3) as xn_pool,
            tc.tile_pool(name="os", bufs=3) as os_pool,
            tc.tile_pool(name="xt", bufs=4) as xt_pool,
            tc.tile_pool(name="ykv", bufs=3) as ykv_pool,
            tc.tile_pool(name="ykn", bufs=3) as ykn_pool,
            tc.tile_pool(name="ct", bufs=3) as ct_pool,
            tc.tile_pool(name="ps_xt", bufs=2, space="PSUM") as ps_xt,
            tc.tile_pool(name="ps_mid", bufs=2, space="PSUM") as ps_mid,
            tc.tile_pool(name="ps_ct", bufs=2, space="PSUM") as ps_ct,
            tc.tile_pool(name="ps_o", bufs=2, space="PSUM") as ps_o,
        ):
            wkv_s = cpool.tile([128, 128], DTR, tag="wkv")
            pkv_a_s = cpool.tile([64, 256], DT, tag="pkv_a")
            pkv_b_s = cpool.tile([64, 256], DT, tag="pkv_b")
            qr_a_s = cpool.tile([64, 128], DTR, tag="qr_a")
            qr_b_s = cpool.tile([64, 128], DTR, tag="qr_b")
            mask_a_s = cpool.tile([128, 256], DT, tag="mask_a")
            mask_b_s = cpool.tile([128, 256], DT, tag="mask_b")
            wmapb_s = cpool.tile([33, 256], DTR, tag="wmapb")
            ident_s = cpool.tile([128, 128], DTR, tag="ident")
            # two manually double-buffered D^T staging tiles; row 32 = ones
            dts0 = cpool.tile([33, 256], DTR, tag="dts0")
            dts1 = cpool.tile([33, 256], DTR, tag="dts1")

            def load_consts():
                pairs = [(ident_s, ident_d), (wkv_s, wkv_d), (pkv_a_s, pkv_a_d),
                         (qr_a_s, qr_a_d), (mask_a_s, mask_a_d),
                         (wmapb_s, wmapb_d), (pkv_b_s, pkv_b_d),
                         (qr_b_s, qr_b_d), (mask_b_s, mask_b_d)]
                for j, (t, d) in enumerate(pairs):
                    eng = nc.sync if j % 2 == 0 else nc.scalar
                    eng.dma_start(out=t[:], in_=d[:])
                nc.sync.dma_start(out=dts0[32:33, :], in_=ones_d[:])
                nc.scalar.dma_start(out=dts1[32:33, :], in_=ones_d[:])

            pair_idx = [0]

            def pair2(xn16, os16, pp, pkv, qr, mask):
                """Two window-pairs emitted stage-interleaved (SW pipelining).

                ctk bank: ct=[*,0:256], ykn_ps=[*,256:320];
                mid bank: ykv=[0:64,0:256], dt=[0:32,256:512]."""
                cs = []
                for p in (pp, pp + 1):
                    dts = dts0 if (pair_idx[0] % 2 == 0) else dts1
                    pair_idx[0] += 1
                    cs.append({'p': p, 'dts': dts})
                for c in cs:
                    p = c['p']
                    xw = [xn16[:, 512 * p + 256 * i:512 * p + 256 * (i + 1)]
                          for i in range(2)]
                    t_xtps = ps_xt.tile([128, 512], DTR, tag="ps_xt")
                    c['xt_ps'] = t_xtps
                    for i in range(2):
                        nc.tensor.transpose(c['xt_ps'][:, 128 * i:128 * i + 128],
                                            xw[i][:, 0:128], ident_s[:])
                        nc.tensor.transpose(
                            c['xt_ps'][:, 256 + 128 * i:256 + 128 * i + 128],
                            xw[i][:, 128:256], ident_s[:])
                for c in cs:
                    t_xt = xt_pool.tile([128, 512], DTR, tag="xt")
                    c['xt'] = t_xt
                    nc.scalar.copy(c['xt'][:], c['xt_ps'][:])
                for c in cs:
                    t_mid = ps_mid.tile([64, 512], DT, tag="ps_mid")
                    c['mid'] = t_mid
                    ykv_ps = c['mid'][0:64, 0:256]
                    nc.tensor.matmul(ykv_ps, wkv_s[:, 0:64], c['xt'][:, 0:256],
                                     start=True, stop=False)
                    nc.tensor.matmul(ykv_ps, wkv_s[:, 64:128], c['xt'][:, 256:512],
                                     start=False, stop=True)
                for c in cs:
                    t_ykv = ykv_pool.tile([64, 256], DTR, tag="ykv")
                    c['ykv'] = t_ykv
                    nc.vector.tensor_add(c['ykv'][:], c['mid'][0:64, 0:256], pkv[:])
                for c in cs:
                    t_ctk = ps_ct.tile([128, 320], DT, tag="ps_ct")
                    c['ctk'] = t_ctk
                    ykn_ps = c['ctk'][:, 256:320].bitcast(DTR)
                    for i in range(2):
                        nc.tensor.transpose(ykn_ps[:, 32 * i:32 * i + 32],
                                            c['ykv'][0:32, 128 * i:128 * i + 128],
                                            ident_s[0:32, 0:32])
                for c in cs:
                    t_ykn = ykn_pool.tile([128, 64], DTR, tag="ykn")
                    c['ykn'] = t_ykn
                    nc.vector.tensor_copy(c['ykn'][:],
                                          c['ctk'][:, 256:320].bitcast(DTR))
                for c in cs:
                    for i in range(2):
                        nc.tensor.matmul(c['ctk'][:, 128 * i:128 * i + 128],
                                         c['ykv'][32:64, 128 * i:128 * i + 128],
                                         qr[32:64, :], start=True, stop=True)
                for c in cs:
                    t_ct = ct_pool.tile([128, 256], DTR, tag="ct")
                    c['ct'] = t_ct
                    nc.vector.tensor_mul(c['ct'][:], c['ctk'][:, 0:256], mask[:])
                for c in cs:
                    dt_ps = c['mid'][0:32, 256:512]
                    for i in range(2):
                        nc.tensor.matmul(dt_ps[:, 128 * i:128 * i + 128],
                                         c['ykn'][:, 32 * i:32 * i + 32],
                                         c['ct'][:, 128 * i:128 * i + 128],
                                         start=True, stop=True)
                for c in cs:
                    nc.vector.tensor_copy(c['dts'][0:32, :],
                                          c['mid'][0:32, 256:512])
                for c in cs:
                    t_ops = ps_o.tile([128, 512], DT, tag="ps_o")
                    c['o_ps'] = t_ops
                    for i in range(2):
                        nc.tensor.matmul(c['o_ps'][:, 256 * i:256 * i + 256],
                                         c['dts'][:, 128 * i:128 * i + 128],
                                         wmapb_s[:], start=True, stop=True)
                for c in cs:
                    nc.scalar.copy(os16[:, 512 * c['p']:512 * c['p'] + 512],
                                   c['o_ps'][:])

            def load_batch(in4, fine_in=False):
                nsi = in4.shape[0]
                pr = 128 // nsi            # rows (partitions) per segment
                xn16 = xn_pool.tile([128, 256 * WB], DTR, tag="xn")
                xv = xn16[:].rearrange("p (wi c) -> p wi c", c=256)
                nic = 4 if fine_in else 1
                wic = WB // nic
                for k in range(nic):
                    for s in range(nsi):
                        eng = nc.sync if s % 2 == 0 else nc.scalar
                        eng.dma_start(out=xv[pr * s:pr * (s + 1),
                                             wic * k:wic * (k + 1), :],
                                      in_=in4[s, :, wic * k:wic * (k + 1), :])
                return xn16

            def batch(in4, out4, pkv, qr, mask, fine_out=False, xn16=None):
                """in4/out4: [si, r, wi, c] DRAM views; si segments per window.

                SBUF layout: partition (64/si)*s + r, free 256*wi + c —
                i.e. window wi occupies [128, 256*wi:256*wi+256]."""
                nsi = in4.shape[0]
                pr = 128 // nsi            # rows (partitions) per segment
                if xn16 is None:
                    xn16 = load_batch(in4)
                os16 = os_pool.tile([128, 256 * WB], DT, tag="os")
                for pp in range(0, WB // 2, 2):
                    pair2(xn16, os16, pp, pkv, qr, mask)
                ov = os16[:].rearrange("p (wi c) -> p wi c", c=256)
                nchunk = 4 if fine_out else 1
                wc = WB // nchunk
                for k in range(nchunk):
                    for s in range(nsi):
                        eng = nc.scalar if s % 2 == 0 else nc.sync
                        eng.dma_start(out=out4[s, :, wc * k:wc * (k + 1), :],
                                      in_=ov[pr * s:pr * (s + 1),
                                             wc * k:wc * (k + 1), :])

            first_in = x3[0:2 * WB, 32:96, :].rearrange(
                "(wi si) r c -> si r wi c", si=2)
            xn_first = load_batch(first_in, fine_in=True)
            load_consts()
            for b in range(n_ab):
                sl = x3[2 * WB * b:2 * WB * (b + 1), 32:96, :]
                ol = o3[2 * WB * b:2 * WB * (b + 1), 32:96, :]
                batch(sl.rearrange("(wi si) r c -> si r wi c", si=2),
                      ol.rearrange("(wi si) r c -> si r wi c", si=2),
                      pkv_a_s, qr_a_s, mask_a_s,
                      xn16=(xn_first if b == 0 else None))
            for b in range(n_bb):
                sl = x3[4 * WB * b:4 * WB * (b + 1), 0:32, :]
                ol = o3[4 * WB * b:4 * WB * (b + 1), 0:32, :]
                batch(sl.rearrange("(wi si) r c -> si r wi c", si=4),
                      ol.rearrange("(wi si) r c -> si r wi c", si=4),
                      pkv_b_s, qr_b_s, mask_b_s, fine_out=(b == n_bb - 1))

    nc.compile()
    return nc


_NC_CACHE = {}


def _get_nc(periods):
    if periods not in _NC_CACHE:
        _NC_CACHE[periods] = build_nc(periods)
    return _NC_CACHE[periods]


def kernel(x, batch, Wk, Wv, Wq, Wmap, bmap, trace=False):
    x = np.ascontiguousarray(np.asarray(x, dtype=np.float32))
    assert x.shape == (N_ROWS, DIM), x.shape
    consts = make_consts(np.asarray(Wk, np.float32), np.asarray(Wv, np.float32),
                         np.asarray(Wq, np.float32), np.asarray(Wmap, np.float32),
                         np.asarray(bmap, np.float32))

    from concourse.bass_utils import run_bass_kernel_spmd
    nc = _get_nc(PERIODS_PER_CORE)
    in_maps = []
    for c in range(N_CORES):
        m = {"x": x[c * ROWS_PER_CORE:(c + 1) * ROWS_PER_CORE]}
        m.update(consts)
        in_maps.append(m)
    res = run_bass_kernel_spmd(nc, in_maps, core_ids=list(range(N_CORES)),
                               trace=trace)
    out = np.concatenate([res.results[c]["out"] for c in range(N_CORES)], axis=0)
    if trace:
        kernel.last_exec_time_ns = res.exec_time_ns
    return (out, np.asarray(batch))


# revision 26
# speedup vs baseline: 1.1985x; 1.1985x over previous
"""Trainium2 Bass kernel for nn_AutoEncoder_17695265259991.

Set-transformer autoencoder over 4096 ragged sets (sizes alternating 32/64,
196608 flat rows, DIM=256, W=32), data-parallel over 8 NeuronCores.

Per core: 24576 rows = 256 periods of 96 rows (one 32-set + one 64-set).
Rows are regrouped into 128-row windows holding whole sets only:
  A-windows: two 64-sets   (128 of them per core)
  B-windows: four 32-sets  (64 of them per core)
Windows are processed in pairs (shared PSUM banks, paired evacuations) and
DMAed in batches of 16 windows (2 MiB per transfer).

Per-window pipeline (all f32, PSUM accumulation):
  x [128,256] --PE transpose--> xT [256p,128] --mm Wkv--> [Yk^T;Yv^T] [64,128]
  (+ sinusoid-PE pattern folded in as a constant added during PSUM evacuation)
  C^T = Yv^T.T @ Q^T(pos-pattern)  -> masked block-diagonal (constant mask)
  D^T = Yk_nat.T @ C^T ; out = [D^T;1].T @ [Wmap^T;bmap]  -> [128,256] rows
"""

import numpy as np

DIM = 256
W = 32
MAXN = 64
N_CORES = 8
N_ROWS = 196608
PERIOD = 96                      # 32-set rows + 64-set rows
ROWS_PER_CORE = N_ROWS // N_CORES          # 24576
PERIODS_PER_CORE = ROWS_PER_CORE // PERIOD  # 256
WB = 16                          # windows per DMA batch


def sinusoid_pe(length, dim):
    pos = np.arange(length, dtype=np.float32)[:, None]
    i = np.arange(dim)[None, :]
    angle = pos / np.power(10000.0, (2.0 * (i // 2)).astype(np.float32) / dim)
    return np.where(i % 2 == 0, np.sin(angle), np.cos(angle)).astype(np.float32)


def _block_diag_mask(sizes):
    R = sum(sizes)
    m = np.zeros((R, R), np.float32)
    off = 0
    for n in sizes:
        m[off:off + n, off:off + n] = 1.0
        off += n
    return m


def make_consts(Wk, Wv, Wq, Wmap, bmap):
    """Host-side packing of the tiny weight-derived constants."""
    pe = sinusoid_pe(MAXN, DIM)
    wkv = np.concatenate([Wk.T, Wv.T], axis=1).astype(np.float32)     # [256, 64]
    # SBUF layout [128, 128]: cols 0:64 = dims 0:128, cols 64:128 = dims 128:256
    wkv_s = np.concatenate([wkv[0:128], wkv[128:256]], axis=1)        # [128, 128]
    Q = (pe @ Wq.T).astype(np.float32)                                # [64, 32]
    pos_a = np.concatenate([np.arange(64), np.arange(64)])
    pos_b = np.concatenate([np.arange(32)] * 4)
    pkv_a = (pe[pos_a] @ wkv).T.astype(np.float32)                    # [64, 128]
    pkv_b = (pe[pos_b] @ wkv).T.astype(np.float32)
    # Q^T patterns live at partitions 32:64 so their base partition matches
    # the Yv^T slice of the stacked [Yk^T;Yv^T] tile used as matmul lhsT.
    qr_a = np.zeros((64, 128), np.float32)
    qr_a[32:64] = Q[pos_a].T
    qr_b = np.zeros((64, 128), np.float32)
    qr_b[32:64] = Q[pos_b].T
    mask_a = _block_diag_mask([64, 64])
    mask_b = _block_diag_mask([32, 32, 32, 32])
    wmapb = np.concatenate([Wmap.T, bmap[None]], axis=0).astype(np.float32)  # [33, 256]
    ident = np.eye(128, dtype=np.float32)
    ones = np.ones((1, 256), dtype=np.float32)
    # pkv/mask are stored pair-wide (2 windows side by side)
    return dict(wkv=wkv_s,
                pkv_a=np.tile(pkv_a, (1, 2)), pkv_b=np.tile(pkv_b, (1, 2)),
                qr_a=qr_a, qr_b=qr_b,
                mask_a=np.tile(mask_a, (1, 2)), mask_b=np.tile(mask_b, (1, 2)),
                wmapb=wmapb, ident=ident, ones=ones)


def build_nc(periods=PERIODS_PER_CORE):
    """Build the per-core Bass program. `periods` must be divisible by 4*WB."""
    import concourse.bass as bass
    import concourse.mybir as mybir
    from concourse import bacc, tile

    DT = mybir.dt.float32
    DTR = mybir.dt.float32r
    assert periods % (4 * WB) == 0, periods
    rows = periods * PERIOD
    n_ab = periods // 2 // WB      # A batches (WB windows each)
    n_bb = periods // 4 // WB      # B batches

    nc = bacc.Bacc()
    x_d = nc.dram_tensor("x", (rows, DIM), DTR, kind="ExternalInput")
    out_d = nc.dram_tensor("out", (rows, DIM), DT, kind="ExternalOutput")
    wkv_d = nc.dram_tensor("wkv", (128, 128), DTR, kind="ExternalInput")
    pkv_a_d = nc.dram_tensor("pkv_a", (64, 256), DT, kind="ExternalInput")
    pkv_b_d = nc.dram_tensor("pkv_b", (64, 256), DT, kind="ExternalInput")
    qr_a_d = nc.dram_tensor("qr_a", (64, 128), DTR, kind="ExternalInput")
    qr_b_d = nc.dram_tensor("qr_b", (64, 128), DTR, kind="ExternalInput")
    mask_a_d = nc.dram_tensor("mask_a", (128, 256), DT, kind="ExternalInput")
    mask_b_d = nc.dram_tensor("mask_b", (128, 256), DT, kind="ExternalInput")
    wmapb_d = nc.dram_tensor("wmapb", (33, 256), DTR, kind="ExternalInput")
    ident_d = nc.dram_tensor("ident", (128, 128), DTR, kind="ExternalInput")
    ones_d = nc.dram_tensor("ones", (1, 256), DTR, kind="ExternalInput")

    x3 = x_d.rearrange("(n r) c -> n r c", r=PERIOD)
    o3 = out_d.rearrange("(n r) c -> n r c", r=PERIOD)

    with tile.TileContext(nc) as tc:
        with (
            tc.tile_pool(name="const", bufs=1) as cpool,
            tc.tile_pool(name="xn", bufs=3) as xn_pool,
            tc.tile_pool(name="os", bufs=3) as os_pool,
            tc.tile_pool(name="xt", bufs=4) as xt_pool,
            tc.tile_pool(name="ykv", bufs=3) as ykv_pool,
            tc.tile_pool(name="ykn", bufs=3) as ykn_pool,
            tc.tile_pool(name="ct", bufs=3) as ct_pool,
            tc.tile_pool(name="ps_xt", bufs=2, space="PSUM") as ps_xt,
            tc.tile_pool(name="ps_mid", bufs=2, space="PSUM") as ps_mid,
            tc.tile_pool(name="ps_ykn", bufs=1, space="PSUM") as ps_ykn,
            tc.tile_pool(name="ps_ct", bufs=2, space="PSUM") as ps_ct,
            tc.tile_pool(name="ps_o", bufs=1, space="PSUM") as ps_o,
        ):
            wkv_s = cpool.tile([128, 128], DTR, tag="wkv")
            pkv_a_s = cpool.tile([64, 256], DT, tag="pkv_a")
            pkv_b_s = cpool.tile([64, 256], DT, tag="pkv_b")
            qr_a_s = cpool.tile([64, 128], DTR, tag="qr_a")
            qr_b_s = cpool.tile([64, 128], DTR, tag="qr_b")
            mask_a_s = cpool.tile([128, 256], DT, tag="mask_a")
            mask_b_s = cpool.tile([128, 256], DT, tag="mask_b")
            wmapb_s = cpool.tile([33, 256], DTR, tag="wmapb")
            ident_s = cpool.tile([128, 128], DTR, tag="ident")
            # two manually double-buffered D^T staging tiles; row 32 = ones
            dts0 = cpool.tile([33, 256], DTR, tag="dts0")
            dts1 = cpool.tile([33, 256], DTR, tag="dts1")

            def load_consts():
                pairs = [(ident_s, ident_d), (wkv_s, wkv_d), (pkv_a_s, pkv_a_d),
                         (qr_a_s, qr_a_d), (mask_a_s, mask_a_d),
                         (wmapb_s, wmapb_d), (pkv_b_s, pkv_b_d),
                         (qr_b_s, qr_b_d), (mask_b_s, mask_b_d)]
                for j, (t, d) in enumerate(pairs):
                    eng = nc.sync if j % 2 == 0 else nc.scalar
                    eng.dma_start(out=t[:], in_=d[:])
                nc.sync.dma_start(out=dts0[32:33, :], in_=ones_d[:])
                nc.scalar.dma_start(out=dts1[32:33, :], in_=ones_d[:])

            pair_idx = [0]

            def pair(xn16, os16, p, pkv, qr, mask):
                """Process windows 2p, 2p+1 of the current WB-window batch."""
                dts = dts0 if (pair_idx[0] % 2 == 0) else dts1
                pair_idx[0] += 1
                xw = [xn16[:, 512 * p + 256 * i:512 * p + 256 * (i + 1)]
                      for i in range(2)]

                # xt layout is chunk-major: [c0w0|c0w1|c1w0|c1w1] so the
                # ykv matmul streams both windows in one Nf=256 f32r matmul
                xt_ps = ps_xt.tile([128, 512], DTR, tag="ps_xt")
                for i in range(2):
                    nc.tensor.transpose((xt_ps[:, 128 * i:128 * i + 128]),
                                        (xw[i][:, 0:128]), (ident_s[:]))
                    nc.tensor.transpose((xt_ps[:, 256 + 128 * i:256 + 128 * i + 128]),
                                        (xw[i][:, 128:256]), (ident_s[:]))
                xt = xt_pool.tile([128, 512], DTR, tag="xt")
                nc.scalar.copy(xt[:], xt_ps[:])

                # mid bank: ykv=[0:64,0:256], dt=[0:32,256:512]
                mid = ps_mid.tile([64, 512], DT, tag="ps_mid")
                ykv_ps = mid[0:64, 0:256]
                nc.tensor.matmul(ykv_ps, (wkv_s[:, 0:64]), (xt[:, 0:256]),
                                 start=True, stop=False)
                nc.tensor.matmul(ykv_ps, (wkv_s[:, 64:128]), (xt[:, 256:512]),
                                 start=False, stop=True)
                ykv = ykv_pool.tile([64, 256], DTR, tag="ykv")
                nc.vector.tensor_add(ykv[:], ykv_ps, pkv[:])

                ykn_ps = ps_ykn.tile([128, 64], DTR, tag="ps_ykn")
                for i in range(2):
                    nc.tensor.transpose((ykn_ps[:, 32 * i:32 * i + 32]),
                                        (ykv[0:32, 128 * i:128 * i + 128]),
                                        (ident_s[0:32, 0:32]))
                ykn = ykn_pool.tile([128, 64], DTR, tag="ykn")
                nc.vector.tensor_copy(ykn[:], ykn_ps[:])

                ct_ps = ps_ct.tile([128, 256], DT, tag="ps_ct")
                for i in range(2):
                    nc.tensor.matmul(ct_ps[:, 128 * i:128 * i + 128],
                                     (ykv[32:64, 128 * i:128 * i + 128]),
                                     (qr[32:64, :]), start=True, stop=True)
                ct = ct_pool.tile([128, 256], DTR, tag="ct")
                nc.vector.tensor_mul(ct[:], ct_ps[:], mask[:])

                dt_ps = mid[0:32, 256:512]
                for i in range(2):
                    nc.tensor.matmul(dt_ps[:, 128 * i:128 * i + 128],
                                     (ykn[:, 32 * i:32 * i + 32]),
                                     (ct[:, 128 * i:128 * i + 128]),
                                     start=True, stop=True)
                nc.vector.tensor_copy(dts[0:32, :], dt_ps[:, :])

                o_ps = ps_o.tile([128, 512], DT, tag="ps_o")
                for i in range(2):
                    nc.tensor.matmul(o_ps[:, 256 * i:256 * i + 256],
                                     (dts[:, 128 * i:128 * i + 128]),
                                     (wmapb_s[:]), start=True, stop=True)
                nc.scalar.copy(os16[:, 512 * p:512 * p + 512], o_ps[:])

            def load_batch(in4, fine_in=False):
                nsi = in4.shape[0]
                pr = 128 // nsi            # rows (partitions) per segment
                xn16 = xn_pool.tile([128, 256 * WB], DTR, tag="xn")
                xv = xn16[:].rearrange("p (wi c) -> p wi c", c=256)
                nic = 4 if fine_in else 1
                wic = WB // nic
                for k in range(nic):
                    for s in range(nsi):
                        eng = nc.sync if s % 2 == 0 else nc.scalar
                        eng.dma_start(out=xv[pr * s:pr * (s + 1),
                                             wic * k:wic * (k + 1), :],
                                      in_=in4[s, :, wic * k:wic * (k + 1), :])
                return xn16

            def batch(in4, out4, pkv, qr, mask, fine_out=False, xn16=None):
                """in4/out4: [si, r, wi, c] DRAM views; si segments per window.

                SBUF layout: partition (64/si)*s + r, free 256*wi + c —
                i.e. window wi occupies [128, 256*wi:256*wi+256]."""
                nsi = in4.shape[0]
                pr = 128 // nsi            # rows (partitions) per segment
                if xn16 is None:
                    xn16 = load_batch(in4)
                os16 = os_pool.tile([128, 256 * WB], DT, tag="os")
                for p in range(WB // 2):
                    pair(xn16, os16, p, pkv, qr, mask)
                ov = os16[:].rearrange("p (wi c) -> p wi c", c=256)
                nchunk = 4 if fine_out else 1
                wc = WB // nchunk
                for k in range(nchunk):
                    for s in range(nsi):
                        eng = nc.scalar if s % 2 == 0 else nc.sync
                        eng.dma_start(out=out4[s, :, wc * k:wc * (k + 1), :],
                                      in_=ov[pr * s:pr * (s + 1),
                                             wc * k:wc * (k + 1), :])

            first_in = x3[0:2 * WB, 32:96, :].rearrange(
                "(wi si) r c -> si r wi c", si=2)
            xn_first = load_batch(first_in, fine_in=True)
            load_consts()
            for b in range(n_ab):
                sl = x3[2 * WB * b:2 * WB * (b + 1), 32:96, :]
                ol = o3[2 * WB * b:2 * WB * (b + 1), 32:96, :]
                batch(sl.rearrange("(wi si) r c -> si r wi c", si=2),
                      ol.rearrange("(wi si) r c -> si r wi c", si=2),
                      pkv_a_s, qr_a_s, mask_a_s,
                      xn16=(xn_first if b == 0 else None))
            for b in range(n_bb):
                sl = x3[4 * WB * b:4 * WB * (b + 1), 0:32, :]
                ol = o3[4 * WB * b:4 * WB * (b + 1), 0:32, :]
                batch(sl.rearrange("(wi si) r c -> si r wi c", si=4),
                      ol.rearrange("(wi si) r c -> si r wi c", si=4),
                      pkv_b_s, qr_b_s, mask_b_s, fine_out=(b == n_bb - 1))

    nc.compile()
    return nc


_NC_CACHE = {}


def _get_nc(periods):
    if periods not in _NC_CACHE:
        _NC_CACHE[periods] = build_nc(periods)
    return _NC_CACHE[periods]


def kernel(x, batch, Wk, Wv, Wq, Wmap, bmap, trace=False):
    x = np.ascontiguousarray(np.asarray(x, dtype=np.float32))
    assert x.shape == (N_ROWS, DIM), x.shape
    consts = make_consts(np.asarray(Wk, np.float32), np.asarray(Wv, np.float32),
                         np.asarray(Wq, np.float32), np.asarray(Wmap, np.float32),
                         np.asarray(bmap, np.float32))

    from concourse.bass_utils import run_bass_kernel_spmd
    nc = _get_nc(PERIODS_PER_CORE)
    in_maps = []
    for c in range(N_CORES):
        m = {"x": x[c * ROWS_PER_CORE:(c + 1) * ROWS_PER_CORE]}
        m.update(consts)
        in_maps.append(m)
    res = run_bass_kernel_spmd(nc, in_maps, core_ids=list(range(N_CORES)),
                               trace=trace)
    out = np.concatenate([res.results[c]["out"] for c in range(N_CORES)], axis=0)
    if trace:
        kernel.last_exec_time_ns = res.exec_time_ns
    return (out, np.asarray(batch))


# revision 27
# speedup vs baseline: 1.1988x; 1.0002x over previous
"""Trainium2 Bass kernel for nn_AutoEncoder_17695265259991.

Set-transformer autoencoder over 4096 ragged sets (sizes alternating 32/64,
196608 flat rows, DIM=256, W=32), data-parallel over 8 NeuronCores.

Per core: 24576 rows = 256 periods of 96 rows (one 32-set + one 64-set).
Rows are regrouped into 128-row windows holding whole sets only:
  A-windows: two 64-sets   (128 of them per core)
  B-windows: four 32-sets  (64 of them per core)
Windows are processed in pairs (shared PSUM banks, paired evacuations) and
DMAed in batches of 16 windows (2 MiB per transfer).

Per-window pipeline (all f32, PSUM accumulation):
  x [128,256] --PE transpose--> xT [256p,128] --mm Wkv--> [Yk^T;Yv^T] [64,128]
  (+ sinusoid-PE pattern folded in as a constant added during PSUM evacuation)
  C^T = Yv^T.T @ Q^T(pos-pattern)  -> masked block-diagonal (constant mask)
  D^T = Yk_nat.T @ C^T ; out = [D^T;1].T @ [Wmap^T;bmap]  -> [128,256] rows
"""

import numpy as np

DIM = 256
W = 32
MAXN = 64
N_CORES = 8
N_ROWS = 196608
PERIOD = 96                      # 32-set rows + 64-set rows
ROWS_PER_CORE = N_ROWS // N_CORES          # 24576
PERIODS_PER_CORE = ROWS_PER_CORE // PERIOD  # 256
WB = 16                          # windows per DMA batch


def sinusoid_pe(length, dim):
    pos = np.arange(length, dtype=np.float32)[:, None]
    i = np.arange(dim)[None, :]
    angle = pos / np.power(10000.0, (2.0 * (i // 2)).astype(np.float32) / dim)
    return np.where(i % 2 == 0, np.sin(angle), np.cos(angle)).astype(np.float32)


def _block_diag_mask(sizes):
    R = sum(sizes)
    m = np.zeros((R, R), np.float32)
    off = 0
    for n in sizes:
        m[off:off + n, off:off + n] = 1.0
        off += n
    return m


def make_consts(Wk, Wv, Wq, Wmap, bmap):
    """Host-side packing of the tiny weight-derived constants."""
    pe = sinusoid_pe(MAXN, DIM)
    wkv = np.concatenate([Wk.T, Wv.T], axis=1).astype(np.float32)     # [256, 64]
    # SBUF layout [128, 128]: cols 0:64 = dims 0:128, cols 64:128 = dims 128:256
    wkv_s = np.concatenate([wkv[0:128], wkv[128:256]], axis=1)        # [128, 128]
    Q = (pe @ Wq.T).astype(np.float32)                                # [64, 32]
    pos_a = np.concatenate([np.arange(64), np.arange(64)])
    pos_b = np.concatenate([np.arange(32)] * 4)
    pkv_a = (pe[pos_a] @ wkv).T.astype(np.float32)                    # [64, 128]
    pkv_b = (pe[pos_b] @ wkv).T.astype(np.float32)
    # Q^T patterns live at partitions 32:64 so their base partition matches
    # the Yv^T slice of the stacked [Yk^T;Yv^T] tile used as matmul lhsT.
    qr_a = np.zeros((64, 128), np.float32)
    qr_a[32:64] = Q[pos_a].T
    qr_b = np.zeros((64, 128), np.float32)
    qr_b[32:64] = Q[pos_b].T
    mask_a = _block_diag_mask([64, 64])
    mask_b = _block_diag_mask([32, 32, 32, 32])
    wmapb = np.concatenate([Wmap.T, bmap[None]], axis=0).astype(np.float32)  # [33, 256]
    ident = np.eye(128, dtype=np.float32)
    ones = np.ones((1, 256), dtype=np.float32)
    # pkv/mask are stored pair-wide (2 windows side by side)
    return dict(wkv=wkv_s,
                pkv_a=np.tile(pkv_a, (1, 2)), pkv_b=np.tile(pkv_b, (1, 2)),
                qr_a=qr_a, qr_b=qr_b,
                mask_a=np.tile(mask_a, (1, 2)), mask_b=np.tile(mask_b, (1, 2)),
                wmapb=wmapb, ident=ident, ones=ones)


def build_nc(periods=PERIODS_PER_CORE):
    """Build the per-core Bass program. `periods` must be divisible by 4*WB."""
    import concourse.bass as bass
    import concourse.mybir as mybir
    from concourse import bacc, tile

    DT = mybir.dt.float32
    DTR = mybir.dt.float32r
    assert periods % (4 * WB) == 0, periods
    rows = periods * PERIOD
    n_ab = periods // 2 // WB      # A batches (WB windows each)
    n_bb = periods // 4 // WB      # B batches

    nc = bacc.Bacc()
    x_d = nc.dram_tensor("x", (rows, DIM), DTR, kind="ExternalInput")
    out_d = nc.dram_tensor("out", (rows, DIM), DT, kind="ExternalOutput")
    wkv_d = nc.dram_tensor("wkv", (128, 128), DTR, kind="ExternalInput")
    pkv_a_d = nc.dram_tensor("pkv_a", (64, 256), DT, kind="ExternalInput")
    pkv_b_d = nc.dram_tensor("pkv_b", (64, 256), DT, kind="ExternalInput")
    qr_a_d = nc.dram_tensor("qr_a", (64, 128), DTR, kind="ExternalInput")
    qr_b_d = nc.dram_tensor("qr_b", (64, 128), DTR, kind="ExternalInput")
    mask_a_d = nc.dram_tensor("mask_a", (128, 256), DT, kind="ExternalInput")
    mask_b_d = nc.dram_tensor("mask_b", (128, 256), DT, kind="ExternalInput")
    wmapb_d = nc.dram_tensor("wmapb", (33, 256), DTR, kind="ExternalInput")
    ident_d = nc.dram_tensor("ident", (128, 128), DTR, kind="ExternalInput")
    ones_d = nc.dram_tensor("ones", (1, 256), DTR, kind="ExternalInput")

    x3 = x_d.rearrange("(n r) c -> n r c", r=PERIOD)
    o3 = out_d.rearrange("(n r) c -> n r c", r=PERIOD)

    with tile.TileContext(nc) as tc:
        with (
            tc.tile_pool(name="const", bufs=1) as cpool,
            tc.tile_pool(name="xn", bufs=4) as xn_pool,
            tc.tile_pool(name="os", bufs=4) as os_pool,
            tc.tile_pool(name="xt", bufs=6) as xt_pool,
            tc.tile_pool(name="ykv", bufs=5) as ykv_pool,
            tc.tile_pool(name="ykn", bufs=5) as ykn_pool,
            tc.tile_pool(name="ct", bufs=5) as ct_pool,
            tc.tile_pool(name="ps_xt", bufs=2, space="PSUM") as ps_xt,
            tc.tile_pool(name="ps_mid", bufs=2, space="PSUM") as ps_mid,
            tc.tile_pool(name="ps_ykn", bufs=1, space="PSUM") as ps_ykn,
            tc.tile_pool(name="ps_ct", bufs=2, space="PSUM") as ps_ct,
            tc.tile_pool(name="ps_o", bufs=1, space="PSUM") as ps_o,
        ):
            wkv_s = cpool.tile([128, 128], DTR, tag="wkv")
            pkv_a_s = cpool.tile([64, 256], DT, tag="pkv_a")
            pkv_b_s = cpool.tile([64, 256], DT, tag="pkv_b")
            qr_a_s = cpool.tile([64, 128], DTR, tag="qr_a")
            qr_b_s = cpool.tile([64, 128], DTR, tag="qr_b")
            mask_a_s = cpool.tile([128, 256], DT, tag="mask_a")
            mask_b_s = cpool.tile([128, 256], DT, tag="mask_b")
            wmapb_s = cpool.tile([33, 256], DTR, tag="wmapb")
            ident_s = cpool.tile([128, 128], DTR, tag="ident")
            # two manually double-buffered D^T staging tiles; row 32 = ones
            dts0 = cpool.tile([33, 256], DTR, tag="dts0")
            dts1 = cpool.tile([33, 256], DTR, tag="dts1")

            def load_consts():
                pairs = [(ident_s, ident_d), (wkv_s, wkv_d), (pkv_a_s, pkv_a_d),
                         (qr_a_s, qr_a_d), (mask_a_s, mask_a_d),
                         (wmapb_s, wmapb_d), (pkv_b_s, pkv_b_d),
                         (qr_b_s, qr_b_d), (mask_b_s, mask_b_d)]
                for j, (t, d) in enumerate(pairs):
                    eng = nc.sync if j % 2 == 0 else nc.scalar
                    eng.dma_start(out=t[:], in_=d[:])
                nc.sync.dma_start(out=dts0[32:33, :], in_=ones_d[:])
                nc.scalar.dma_start(out=dts1[32:33, :], in_=ones_d[:])

            pair_idx = [0]

            def pair(xn16, os16, p, pkv, qr, mask):
                """Process windows 2p, 2p+1 of the current WB-window batch."""
                dts = dts0 if (pair_idx[0] % 2 == 0) else dts1
                pair_idx[0] += 1
                xw = [xn16[:, 512 * p + 256 * i:512 * p + 256 * (i + 1)]
                      for i in range(2)]

                # xt layout is chunk-major: [c0w0|c0w1|c1w0|c1w1] so the
                # ykv matmul streams both windows in one Nf=256 f32r matmul
                xt_ps = ps_xt.tile([128, 512], DTR, tag="ps_xt")
                for i in range(2):
                    nc.tensor.transpose((xt_ps[:, 128 * i:128 * i + 128]),
                                        (xw[i][:, 0:128]), (ident_s[:]))
                    nc.tensor.transpose((xt_ps[:, 256 + 128 * i:256 + 128 * i + 128]),
                                        (xw[i][:, 128:256]), (ident_s[:]))
                xt = xt_pool.tile([128, 512], DTR, tag="xt")
                nc.scalar.copy(xt[:], xt_ps[:])

                # mid bank: ykv=[0:64,0:256], dt=[0:32,256:512]
                mid = ps_mid.tile([64, 512], DT, tag="ps_mid")
                ykv_ps = mid[0:64, 0:256]
                nc.tensor.matmul(ykv_ps, (wkv_s[:, 0:64]), (xt[:, 0:256]),
                                 start=True, stop=False)
                nc.tensor.matmul(ykv_ps, (wkv_s[:, 64:128]), (xt[:, 256:512]),
                                 start=False, stop=True)
                ykv = ykv_pool.tile([64, 256], DTR, tag="ykv")
                nc.vector.tensor_add(ykv[:], ykv_ps, pkv[:])

                ykn_ps = ps_ykn.tile([128, 64], DTR, tag="ps_ykn")
                for i in range(2):
                    nc.tensor.transpose((ykn_ps[:, 32 * i:32 * i + 32]),
                                        (ykv[0:32, 128 * i:128 * i + 128]),
                                        (ident_s[0:32, 0:32]))
                ykn = ykn_pool.tile([128, 64], DTR, tag="ykn")
                nc.vector.tensor_copy(ykn[:], ykn_ps[:])

                ct_ps = ps_ct.tile([128, 256], DT, tag="ps_ct")
                for i in range(2):
                    nc.tensor.matmul(ct_ps[:, 128 * i:128 * i + 128],
                                     (ykv[32:64, 128 * i:128 * i + 128]),
                                     (qr[32:64, :]), start=True, stop=True)
                ct = ct_pool.tile([128, 256], DTR, tag="ct")
                nc.vector.tensor_mul(ct[:], ct_ps[:], mask[:])

                dt_ps = mid[0:32, 256:512]
                for i in range(2):
                    nc.tensor.matmul(dt_ps[:, 128 * i:128 * i + 128],
                                     (ykn[:, 32 * i:32 * i + 32]),
                                     (ct[:, 128 * i:128 * i + 128]),
                                     start=True, stop=True)
                nc.vector.tensor_copy(dts[0:32, :], dt_ps[:, :])

                o_ps = ps_o.tile([128, 512], DT, tag="ps_o")
                for i in range(2):
                    nc.tensor.matmul(o_ps[:, 256 * i:256 * i + 256],
                                     (dts[:, 128 * i:128 * i + 128]),
                                     (wmapb_s[:]), start=True, stop=True)
                nc.scalar.copy(os16[:, 512 * p:512 * p + 512], o_ps[:])

            def load_batch(in4, fine_in=False):
                nsi = in4.shape[0]
                pr = 128 // nsi            # rows (partitions) per segment
                xn16 = xn_pool.tile([128, 256 * WB], DTR, tag="xn")
                xv = xn16[:].rearrange("p (wi c) -> p wi c", c=256)
                nic = 4 if fine_in else 1
                wic = WB // nic
                for k in range(nic):
                    for s in range(nsi):
                        eng = nc.sync if s % 2 == 0 else nc.scalar
                        eng.dma_start(out=xv[pr * s:pr * (s + 1),
                                             wic * k:wic * (k + 1), :],
                                      in_=in4[s, :, wic * k:wic * (k + 1), :])
                return xn16

            def batch(in4, out4, pkv, qr, mask, fine_out=False, xn16=None):
                """in4/out4: [si, r, wi, c] DRAM views; si segments per window.

                SBUF layout: partition (64/si)*s + r, free 256*wi + c —
                i.e. window wi occupies [128, 256*wi:256*wi+256]."""
                nsi = in4.shape[0]
                pr = 128 // nsi            # rows (partitions) per segment
                if xn16 is None:
                    xn16 = load_batch(in4)
                os16 = os_pool.tile([128, 256 * WB], DT, tag="os")
                for p in range(WB // 2):
                    pair(xn16, os16, p, pkv, qr, mask)
                ov = os16[:].rearrange("p (wi c) -> p wi c", c=256)
                nchunk = 4 if fine_out else 1
                wc = WB // nchunk
                for k in range(nchunk):
                    for s in range(nsi):
                        eng = nc.scalar if s % 2 == 0 else nc.sync
                        eng.dma_start(out=out4[s, :, wc * k:wc * (k + 1), :],
                                      in_=ov[pr * s:pr * (s + 1),
                                             wc * k:wc * (k + 1), :])

            first_in = x3[0:2 * WB, 32:96, :].rearrange(
                "(wi si) r c -> si r wi c", si=2)
            xn_first = load_batch(first_in, fine_in=True)
            load_consts()
            for b in range(n_ab):
                sl = x3[2 * WB * b:2 * WB * (b + 1), 32:96, :]
                ol = o3[2 * WB * b:2 * WB * (b + 1), 32:96, :]
                batch(sl.rearrange("(wi si) r c -> si r wi c", si=2),
                      ol.rearrange("(wi si) r c -> si r wi c", si=2),
                      pkv_a_s, qr_a_s, mask_a_s,
                      xn16=(xn_first if b == 0 else None))
            for b in range(n_bb):
                sl = x3[4 * WB * b:4 * WB * (b + 1), 0:32, :]
                ol = o3[4 * WB * b:4 * WB * (b + 1), 0:32, :]
                batch(sl.rearrange("(wi si) r c -> si r wi c", si=4),
                      ol.rearrange("(wi si) r c -> si r wi c", si=4),
                      pkv_b_s, qr_b_s, mask_b_s, fine_out=(b == n_bb - 1))

    nc.compile()
    return nc


_NC_CACHE = {}


def _get_nc(periods):
    if periods not in _NC_CACHE:
        _NC_CACHE[periods] = build_nc(periods)
    return _NC_CACHE[periods]


def kernel(x, batch, Wk, Wv, Wq, Wmap, bmap, trace=False):
    x = np.ascontiguousarray(np.asarray(x, dtype=np.float32))
    assert x.shape == (N_ROWS, DIM), x.shape
    consts = make_consts(np.asarray(Wk, np.float32), np.asarray(Wv, np.float32),
                         np.asarray(Wq, np.float32), np.asarray(Wmap, np.float32),
                         np.asarray(bmap, np.float32))

    from concourse.bass_utils import run_bass_kernel_spmd
    nc = _get_nc(PERIODS_PER_CORE)
    in_maps = []
    for c in range(N_CORES):
        m = {"x": x[c * ROWS_PER_CORE:(c + 1) * ROWS_PER_CORE]}
        m.update(consts)
        in_maps.append(m)
    res = run_bass_kernel_spmd(nc, in_maps, core_ids=list(range(N_CORES)),
                               trace=trace)
    out = np.concatenate([res.results[c]["out"] for c in range(N_CORES)], axis=0)
    if trace:
        kernel.last_exec_time_ns = res.exec_time_ns
    return (out, np.asarray(batch))
